# revision 7
# baseline (speedup 1.0000x reference)
from collections import deque

import numpy as np
import ml_dtypes

import concourse.mybir as mybir
from concourse import bacc
from concourse.tile import TileContext
from concourse.bass_utils import run_bass_kernel_spmd

H, D, DH = 12, 768, 64
B, S = 2, 2048
NCORES = 8
CORES_PER_BATCH = 4
HPC = 3
SQ = 512
NSQ = S // SQ
NSK = S // 128
NOP = 3

NSK8 = 0
GROUPS = ((0, 4), (4, 6), (6, 10), (10, 12), (12, 16))
assert NSK8 in (0, 4, 6, 10, 12, 16)

F32 = mybir.dt.float32
F32R = mybir.dt.float32r
F16 = mybir.dt.float16
F8 = mybir.dt.float8e4
DR = mybir.MatmulPerfMode.DoubleRow
ADD = mybir.AluOpType.add
MULT = mybir.AluOpType.mult
EXP = mybir.ActivationFunctionType.Exp
E4M3 = ml_dtypes.float8_e4m3fn


def _build_module():
    nc = bacc.Bacc("TRN2", target_bir_lowering=False, debug=False, num_devices=NCORES)
    dram = {}
    def P(name, shape, dt):
        dram[name] = nc.declare_dram_parameter(name, shape, dt, isOutput=False)
    P("x8", [128, NOP, 2, S], F8)
    P("dx8", [128, NOP, 2, S], F8)
    P("x16", [128, NOP, 2, S], F8)
    P("wqk8", [128, HPC, NOP, 2, 128], F8)
    P("wqk16", [128, HPC, NOP, 2, 128], F8)
    P("dwqk8", [128, HPC, NOP, 2, 128], F8)
    P("wv8", [128, NOP, 2, 256], F8)
    P("wv16", [128, NOP, 2, 256], F8)
    P("dwv8", [128, NOP, 2, 256], F8)
    P("bqk", [128, HPC], F32)
    P("bv8", [1, 2, 256], F8)
    P("vones", [1, 2, 128], F8)
    P("wo01", [128, D], F32R)
    P("wo2", [64, D], F32R)
    out = nc.declare_dram_parameter("out", [S, D], F32, isOutput=True)

    with TileContext(nc) as tc:
        _body(nc, tc, dram, out)
    nc.compile()
    return nc


def _body(nc, tc, dram, out):
    with (
        tc.tile_pool(name="persist", bufs=1) as P1,
        tc.tile_pool(name="work", bufs=4) as W2,
        tc.tile_pool(name="pr8", bufs=2) as PR8,
        tc.tile_pool(name="pr16", bufs=2) as PR16,
        tc.tile_pool(name="spsb", bufs=1, space="PSUM") as SPSB,
        tc.tile_pool(name="spss", bufs=1, space="PSUM") as SPSS,
        tc.tile_pool(name="cps", bufs=1, space="PSUM") as CPS,
        tc.tile_pool(name="acc", bufs=1, space="PSUM") as ACC,
    ):
        sb = {}
        for name in ("x8", "dx8", "x16", "wqk8", "wqk16", "dwqk8",
                     "wv8", "wv16", "dwv8", "bqk", "bv8", "vones",
                     "wo01", "wo2"):
            shape = list(dram[name].shape)
            dt = {"bqk": F32, "wo01": F32R, "wo2": F32R}.get(name, F8)
            sb[name] = P1.tile(shape, dt, tag=name, name=name)

        qT = [P1.tile([64, S], F32R, tag=f"qT{h}", name=f"qT{h}")
              for h in range(HPC)]
        kT = [P1.tile([64, S], F32R, tag=f"kT{h}", name=f"kT{h}")
              for h in range(HPC)]
        v8 = dv8 = vp16 = None
        if NSK8:
            v8 = P1.tile([128, NSK8, HPC, 128], F8, tag="v8", name="v8")
            dv8 = P1.tile([128, NSK8, HPC, 128], F8, tag="dv8", name="dv8")
        if NSK8 < NSK:
            vp16 = P1.tile([128, NSK - NSK8, HPC, 128], F16, tag="vp16",
                           name="vp16")

        dma = nc.sync.dma_start
        dma(sb["x8"][:, :, :, 0:SQ], dram["x8"][:, :, :, 0:SQ])
        dma(sb["wqk8"][:, 0], dram["wqk8"][:, 0])
        dma(sb["dx8"][:, :, :, 0:SQ], dram["dx8"][:, :, :, 0:SQ])
        dma(sb["wqk16"][:, 0], dram["wqk16"][:, 0])
        dma(sb["dwqk8"][:, 0], dram["dwqk8"][:, 0])
        dma(sb["x16"][:, :, :, 0:SQ], dram["x16"][:, :, :, 0:SQ])
        dma(sb["bqk"][:], dram["bqk"][:])
        for n in ("wv8", "wv16", "dwv8", "bv8", "vones"):
            dma(sb[n][:], dram[n][:])
        for sc in range(1, NSQ):
            sl = np.s_[:, :, :, sc * SQ:(sc + 1) * SQ]
            dma(sb["x8"][sl], dram["x8"][sl])
            dma(sb["dx8"][sl], dram["dx8"][sl])
            dma(sb["x16"][sl], dram["x16"][sl])
        dma(sb["wqk8"][:, 1:3], dram["wqk8"][:, 1:3])
        dma(sb["wqk16"][:, 1:3], dram["wqk16"][:, 1:3])
        dma(sb["dwqk8"][:, 1:3], dram["dwqk8"][:, 1:3])
        dma(sb["wo01"][:], dram["wo01"][:])
        dma(sb["wo2"][:], dram["wo2"][:])
        if v8 is not None:
            nc.gpsimd.memset(v8[:, :, :, 64:128], 1.0)
            nc.gpsimd.memset(dv8[:, :, :, 64:128], 0.0)
        if vp16 is not None:
            nc.gpsimd.memset(vp16[:, :, :, 64:128], 1.0)

        def qk_unit(h, sc):
            ps = ACC.tile([128, SQ], F32, tag="acc", name=f"qkps{h}_{sc}")
            s0 = sc * SQ
            passes = (
                (sb["wqk8"], sb["x8"]),
                (sb["wqk16"], sb["dx8"]),
                (sb["dwqk8"], sb["x16"]),
            )
            for pi, (wt, xt) in enumerate(passes):
                for j in range(NOP):
                    nc.tensor.matmul(
                        ps[:],
                        wt[:, h, j],
                        xt[:, j, :, s0:s0 + SQ],
                        start=(pi == 0 and j == 0),
                        stop=(pi == 2 and j == NOP - 1),
                        perf_mode=DR,
                    )
            nc.vector.tensor_tensor(
                qT[h][:, s0:s0 + SQ], ps[0:64, :],
                sb["bqk"][0:64, h:h + 1].to_broadcast([64, SQ]), ADD)
            nc.vector.tensor_tensor(
                kT[h][:, s0:s0 + SQ], ps[64:128, :],
                sb["bqk"][64:128, h:h + 1].to_broadcast([64, SQ]), ADD)

        def v_unit(k):
            ps = ACC.tile([128, 256], F32, tag="acc", name=f"vps{k}")
            c0 = k * 128
            passes = (
                (sb["x8"], sb["wv8"]),
                (sb["dx8"], sb["wv16"]),
                (sb["x16"], sb["dwv8"]),
            )
            for pi, (xt, wt) in enumerate(passes):
                for j in range(NOP):
                    nc.tensor.matmul(
                        ps[:],
                        xt[:, j, :, c0:c0 + 128],
                        wt[:, j],
                        start=(pi == 0 and j == 0),
                        stop=False,
                        perf_mode=DR,
                    )
            nc.tensor.matmul(
                ps[:], sb["vones"][:], sb["bv8"][:],
                start=False, stop=True, perf_mode=DR)
            src = ps[:, 0:HPC * 64].rearrange("p (h m) -> p h m", m=64)
            if k < NSK8:
                d8 = v8[:, k, :, 0:64]
                nc.vector.tensor_copy(d8, src)
                nc.vector.scalar_tensor_tensor(
                    dv8[:, k, :, 0:64], d8, -1.0, src, MULT, ADD)
            else:
                nc.vector.tensor_copy(vp16[:, k - NSK8, :, 0:64], src)

        def proj_piece(sc, ms, half, ctx01, ctx2, ot):
            n0, nw = (0, 512) if half == 0 else (512, 256)
            ps = ACC.tile([128, nw], F32, tag="acc", name=f"ops{sc}_{ms}_{half}")
            nc.tensor.matmul(ps[:], ctx01[:, ms * 128:(ms + 1) * 128],
                             sb["wo01"][:, n0:n0 + nw], start=True, stop=False)
            nc.tensor.matmul(ps[:], ctx2[:, ms * 128:(ms + 1) * 128],
                             sb["wo2"][:, n0:n0 + nw], start=False, stop=True)
            nc.vector.tensor_copy(ot[:, n0:n0 + nw], ps[:])
            if half == 1:
                r0 = (sc * 4 + ms) * 128
                nc.sync.dma_start(out[r0:r0 + 128, :], ot[:])

        filler = deque()

        def pop(n=1):
            for _ in range(n):
                if filler:
                    filler.popleft()()

        def attention_block(sc, h, ctx01, ctx2, pops_per_group=1, lag=1,
                            trail_pops=2):
            probs8 = probs16 = None
            if NSK8:
                probs8 = PR8.tile([128, NSK8, SQ], F8, tag="p8",
                                  name=f"p8_{sc}_{h}")
            if NSK8 < NSK:
                probs16 = PR16.tile([128, NSK - NSK8, SQ], F16, tag="p16",
                                    name=f"p16_{sc}_{h}")
            cps = CPS.tile([128, SQ], F32, tag="cps", name=f"cps{sc}_{h}")
            state = {"first": True}

            def probsv_group(gi):
                g0, g1 = GROUPS[gi]
                last_g = gi == len(GROUPS) - 1
                if g1 <= NSK8:
                    for jp in range(g0 // 2, g1 // 2):
                        for t8 in (v8, dv8):
                            nc.tensor.matmul(
                                cps[:],
                                t8[:, 2 * jp:2 * jp + 2, h, :],
                                probs8[:, 2 * jp:2 * jp + 2, :],
                                start=state.pop("first", False),
                                stop=(last_g and t8 is dv8
                                      and jp == g1 // 2 - 1),
                                perf_mode=DR,
                            )
                else:
                    for mk in range(g0, g1):
                        nc.tensor.matmul(
                            cps[:],
                            vp16[:, mk - NSK8, h, :],
                            probs16[:, mk - NSK8, :],
                            start=state.pop("first", False),
                            stop=(last_g and mk == g1 - 1),
                        )

            for gi, (g0, g1) in enumerate(GROUPS):
                n = g1 - g0
                pool = SPSB if n == 4 else SPSS
                sps = pool.tile([128, n * SQ], F32, tag=f"sps{n}",
                                name=f"sps{sc}_{h}_{gi}")
                for mk in range(g0, g1):
                    nc.tensor.matmul(
                        sps[:, (mk - g0) * SQ:(mk - g0 + 1) * SQ],
                        kT[h][:, mk * 128:(mk + 1) * 128],
                        qT[h][:, sc * SQ:(sc + 1) * SQ],
                        start=True, stop=True,
                    )
                dst = (probs8[:, g0:g1] if g1 <= NSK8
                       else probs16[:, g0 - NSK8:g1 - NSK8])
                nc.scalar.activation(dst, sps[:], EXP, scale=0.125)
                pop(pops_per_group)
                if gi >= lag:
                    probsv_group(gi - lag)
            for gi in range(len(GROUPS) - lag, len(GROUPS)):
                if gi == len(GROUPS) - 1:
                    pop(trail_pops)
                probsv_group(gi)
            r = W2.tile([64, SQ], F32, tag="recip", name=f"r{sc}_{h}")
            nc.vector.reciprocal(r[:], cps[64:128, :])
            dst = ctx01[h * 64:(h + 1) * 64, :] if h < 2 else ctx2[:]
            nc.vector.tensor_tensor(dst, cps[0:64, :], r[:], MULT)

        warm = P1.tile([64, 512], F32R, tag="warm")
        nc.vector.memset(warm[:].bitcast(F32), 0.0)
        wps = ACC.tile([128, 512], F32, tag="acc", name="warmps")
        for _ in range(8):
            nc.tensor.matmul(wps[:], warm[:, 0:128], warm[:], start=True, stop=True)
        wact = P1.tile([64, 1], F16, tag="wact")
        nc.scalar.activation(wact[:], warm[:, 0:1].bitcast(F32), EXP, scale=0.125)

        qk_unit(0, 0)

        ctxs = {}
        for sc in range(NSQ):
            ctxs[sc] = (
                W2.tile([128, SQ], F32R, tag="ctx01", name=f"c01_{sc}"),
                W2.tile([64, SQ], F32R, tag="ctx2", name=f"c2_{sc}"),
            )
            for h in range(HPC):
                pops, lag, trail = 1, 1, 2
                if sc == 0 and h == 0:
                    filler.extend([
                        lambda: qk_unit(0, 1), lambda: v_unit(0),
                        lambda: v_unit(1),
                        lambda: v_unit(2), lambda: v_unit(3),
                        lambda: qk_unit(0, 2),
                        lambda: qk_unit(0, 3),
                        lambda: v_unit(4), lambda: v_unit(5),
                        lambda: v_unit(6), lambda: v_unit(7),
                        lambda: v_unit(8), lambda: v_unit(9),
                        lambda: v_unit(10), lambda: v_unit(11),
                        lambda: v_unit(12), lambda: v_unit(13),
                        lambda: v_unit(14), lambda: v_unit(15),
                        lambda: qk_unit(1, 0),
                    ])
                    pops, lag, trail = 3, 2, 5
                elif sc == 0 and h == 1:
                    filler.extend([
                        lambda: qk_unit(1, 1), lambda: qk_unit(1, 2),
                        lambda: qk_unit(1, 3), lambda: qk_unit(2, 0),
                    ])
                elif sc == 0 and h == 2:
                    filler.extend([
                        lambda i=i: qk_unit(2, i) for i in range(1, NSQ)
                    ])
                attention_block(sc, h, *ctxs[sc], pops_per_group=pops,
                                lag=lag, trail_pops=trail)
            ot_tiles = [
                W2.tile([128, D], F32, tag="ot", name=f"ot{sc}_{ms}")
                for ms in range(4)
            ]
            for ms in range(4):
                for half in range(2):
                    filler.append(
                        lambda sc=sc, ms=ms, half=half, ot=ot_tiles[ms]:
                        proj_piece(sc, ms, half, *ctxs[sc], ot)
                    )
        while filler:
            filler.popleft()()


_CACHE = {}


def _get_module():
    if "nc" not in _CACHE:
        _CACHE["nc"] = _build_module()
    return _CACHE["nc"]


def _split8(a):
    a8 = a.astype(E4M3)
    af = a8.astype(np.float32)
    da8 = ((a - af) * 16.0).astype(E4M3)
    a16 = (af / 16.0).astype(E4M3)
    return a8, da8, a16


def _pairchunk(a):
    n = a.shape[1]
    return np.ascontiguousarray(
        a.reshape(NOP, 2, 128, n).transpose(2, 0, 1, 3))


def make_in_maps(x, Wq, Wk, Wv, bq, bk, bv, Wo):
    f = np.float32
    in_maps = []
    vones = np.zeros((1, 2, 128), E4M3)
    vones[0, 0] = 1.0
    vones[0, 1] = 0.0625
    for c in range(NCORES):
        b = c // CORES_PER_BATCH
        hh = [HPC * (c % CORES_PER_BATCH) + i for i in range(HPC)]
        x8, dx8, x16 = (_pairchunk(t) for t in _split8(np.asarray(x[b]).T))
        wqk = np.stack(
            [np.concatenate([Wq[h], Wk[h]], axis=1) for h in hh])
        w8, dw8, w16 = (
            np.stack([_pairchunk(t[i]) for i in range(HPC)], axis=1)
            for t in _split8(wqk))
        wv_stack = np.concatenate(
            [Wv[h] for h in hh] + [np.zeros((D, 64), f)], axis=1)
        v8, dv8, v16 = (_pairchunk(t) for t in _split8(wv_stack))
        bv_cat = np.concatenate([bv[h] for h in hh] + [np.zeros(64, f)])
        bv8 = np.zeros((1, 2, 256), E4M3)
        bv8[0, 0] = bv_cat.astype(E4M3)
        bv8[0, 1] = ((bv_cat - bv8[0, 0].astype(f)) * 16.0).astype(E4M3)
        in_maps.append({
            "x8": x8, "dx8": dx8, "x16": x16,
            "wqk8": w8, "wqk16": w16, "dwqk8": dw8,
            "wv8": v8, "wv16": v16, "dwv8": dv8,
            "bqk": np.ascontiguousarray(
                np.stack([np.concatenate([bq[h], bk[h]]) for h in hh], axis=1)
            ).astype(f, copy=False),
            "bv8": bv8, "vones": vones,
            "wo01": np.ascontiguousarray(
                Wo[hh[0] * DH:(hh[0] + 2) * DH, :]).astype(f, copy=False),
            "wo2": np.ascontiguousarray(
                Wo[hh[2] * DH:(hh[2] + 1) * DH, :]).astype(f, copy=False),
        })
    return in_maps


def gather(results, bo):
    out = np.empty((B, S, D), np.float32)
    for b in range(B):
        acc = results[b * CORES_PER_BATCH]["out"].astype(np.float32, copy=True)
        for c in range(b * CORES_PER_BATCH + 1, (b + 1) * CORES_PER_BATCH):
            acc += results[c]["out"]
        out[b] = acc + bo[None, :].astype(np.float32)
    return out


def kernel(x, Wq, Wk, Wv, bq, bk, bv, Wo, bo, c=0, **_unused):
    x, Wq, Wk, Wv, bq, bk, bv, Wo, bo = (
        np.asarray(a, np.float32) for a in (x, Wq, Wk, Wv, bq, bk, bv, Wo, bo)
    )
    nc = _get_module()
    in_maps = make_in_maps(x, Wq, Wk, Wv, bq, bk, bv, Wo)
    res = run_bass_kernel_spmd(nc, in_maps, list(range(NCORES)))
    return gather(res.results, bo)


# revision 13
# speedup vs baseline: 1.1653x; 1.1653x over previous
from collections import deque

import numpy as np
import ml_dtypes

import concourse.mybir as mybir
from concourse import bacc
from concourse.tile import TileContext
from concourse.bass_utils import run_bass_kernel_spmd

H, D, DH = 12, 768, 64
B, S = 2, 2048
NCORES = 8
CORES_PER_BATCH = 4
HPC = 3
SQ = 512
NSQ = S // SQ
NSK = S // 128
NOP = 3

GROUPS = ((0, 4), (4, 5), (5, 9), (9, 10), (10, 14), (14, 15), (15, 16))
NSK8 = 16
assert NSK8 in (0, 4, 10, 14, 16)
EXPBIAS = -1.5

F32 = mybir.dt.float32
F32R = mybir.dt.float32r
F16 = mybir.dt.float16
F8 = mybir.dt.float8e4
DR = mybir.MatmulPerfMode.DoubleRow
ADD = mybir.AluOpType.add
MULT = mybir.AluOpType.mult
EXP = mybir.ActivationFunctionType.Exp
E4M3 = ml_dtypes.float8_e4m3fn


def _build_module():
    nc = bacc.Bacc("TRN2", target_bir_lowering=False, debug=False, num_devices=NCORES)
    dram = {}
    def P(name, shape, dt):
        dram[name] = nc.declare_dram_parameter(name, shape, dt, isOutput=False)
    P("x8", [128, NOP, 2, S], F8)
    P("dx8", [128, NOP, 2, S], F8)
    P("x16", [128, NOP, 2, S], F8)
    P("wqk8", [128, HPC, NOP, 2, 128], F8)
    P("wqk16", [128, HPC, NOP, 2, 128], F8)
    P("dwqk8", [128, HPC, NOP, 2, 128], F8)
    P("wv8", [128, NOP, 2, 256], F8)
    P("wv16", [128, NOP, 2, 256], F8)
    P("dwv8", [128, NOP, 2, 256], F8)
    P("bqk", [128, HPC], F32)
    P("bv8", [1, 2, 256], F8)
    P("vones", [1, 2, 128], F8)
    P("wo01", [128, D], F32R)
    P("wo2", [64, D], F32R)
    out = nc.declare_dram_parameter("out", [S, D], F32, isOutput=True)

    with TileContext(nc) as tc:
        _body(nc, tc, dram, out)
    nc.compile()
    return nc


def _body(nc, tc, dram, out):
    with (
        tc.tile_pool(name="persist", bufs=1) as P1,
        tc.tile_pool(name="work", bufs=4) as W2,
        tc.tile_pool(name="pr8", bufs=2) as PR8,
        tc.tile_pool(name="pr16", bufs=2) as PR16,
        tc.tile_pool(name="spsb", bufs=1, space="PSUM") as SPSB,
        tc.tile_pool(name="spss", bufs=1, space="PSUM") as SPSS,
        tc.tile_pool(name="cps", bufs=1, space="PSUM") as CPS,
        tc.tile_pool(name="acc", bufs=2, space="PSUM") as ACC,
    ):
        sb = {}
        for name in ("x8", "dx8", "x16", "wqk8", "wqk16", "dwqk8",
                     "wv8", "wv16", "dwv8", "bqk", "bv8", "vones",
                     "wo01", "wo2"):
            shape = list(dram[name].shape)
            dt = {"bqk": F32, "wo01": F32R, "wo2": F32R}.get(name, F8)
            sb[name] = P1.tile(shape, dt, tag=name, name=name)

        qT = [P1.tile([64, S], F32R, tag=f"qT{h}", name=f"qT{h}")
              for h in range(HPC)]
        kT = [P1.tile([64, S], F32R, tag=f"kT{h}", name=f"kT{h}")
              for h in range(HPC)]
        v8 = dv8 = vp16 = None
        if NSK8:
            v8 = P1.tile([128, NSK8, HPC, 128], F8, tag="v8", name="v8")
            dv8 = P1.tile([128, NSK8, HPC, 128], F8, tag="dv8", name="dv8")
        if NSK8 < NSK:
            vp16 = P1.tile([128, NSK - NSK8, HPC, 128], F16, tag="vp16",
                           name="vp16")

        dma = nc.sync.dma_start
        dma(sb["x8"][:, :, :, 0:SQ], dram["x8"][:, :, :, 0:SQ])
        dma(sb["wqk8"][:, 0], dram["wqk8"][:, 0])
        dma(sb["dx8"][:, :, :, 0:SQ], dram["dx8"][:, :, :, 0:SQ])
        dma(sb["wqk16"][:, 0], dram["wqk16"][:, 0])
        dma(sb["dwqk8"][:, 0], dram["dwqk8"][:, 0])
        dma(sb["x16"][:, :, :, 0:SQ], dram["x16"][:, :, :, 0:SQ])
        dma(sb["bqk"][:], dram["bqk"][:])

        def dma_x(sc):
            sl = np.s_[:, :, :, sc * SQ:(sc + 1) * SQ]
            dma(sb["x8"][sl], dram["x8"][sl])
            dma(sb["dx8"][sl], dram["dx8"][sl])
            dma(sb["x16"][sl], dram["x16"][sl])

        dma_x(1)
        for n in ("wv8", "wv16", "dwv8", "bv8", "vones"):
            dma(sb[n][:], dram[n][:])
        dma_x(2)
        dma(sb["wqk8"][:, 1:3], dram["wqk8"][:, 1:3])
        dma(sb["wqk16"][:, 1:3], dram["wqk16"][:, 1:3])
        dma(sb["dwqk8"][:, 1:3], dram["dwqk8"][:, 1:3])
        dma_x(3)
        dma(sb["wo01"][:], dram["wo01"][:])
        dma(sb["wo2"][:], dram["wo2"][:])
        ebias = P1.tile([128, 1], F32, tag="ebias")
        nc.vector.memset(ebias[:], EXPBIAS)
        if v8 is not None:
            nc.gpsimd.memset(v8[:, :, :, 64:128], 1.0)
            nc.gpsimd.memset(dv8[:, :, :, 64:128], 0.0)
        if vp16 is not None:
            nc.gpsimd.memset(vp16[:, :, :, 64:128], 1.0)


        def qk_unit(h, sc):
            ps = ACC.tile([128, SQ], F32, tag="acc", name=f"qkps{h}_{sc}")
            s0 = sc * SQ
            passes = (
                (sb["wqk8"], sb["x8"]),
                (sb["wqk16"], sb["dx8"]),
                (sb["dwqk8"], sb["x16"]),
            )
            def m():
                for pi, (wt, xt) in enumerate(passes):
                    for j in range(NOP):
                        nc.tensor.matmul(
                            ps[:], wt[:, h, j], xt[:, j, :, s0:s0 + SQ],
                            start=(pi == 0 and j == 0),
                            stop=(pi == 2 and j == NOP - 1),
                            perf_mode=DR,
                        )
            def e():
                nc.vector.scalar_tensor_tensor(
                    qT[h][:, s0:s0 + SQ], ps[0:64, :], 0.0625,
                    sb["bqk"][0:64, h:h + 1].to_broadcast([64, SQ]),
                    MULT, ADD)
                nc.vector.scalar_tensor_tensor(
                    kT[h][:, s0:s0 + SQ], ps[64:128, :], 0.0625,
                    sb["bqk"][64:128, h:h + 1].to_broadcast([64, SQ]),
                    MULT, ADD)
            return m, e

        def v_unit(k):
            ps = ACC.tile([128, 256], F32, tag="acc", name=f"vps{k}")
            c0 = k * 128
            passes = (
                (sb["x8"], sb["wv8"]),
                (sb["dx8"], sb["wv16"]),
                (sb["x16"], sb["dwv8"]),
            )
            def m():
                for pi, (xt, wt) in enumerate(passes):
                    for j in range(NOP):
                        nc.tensor.matmul(
                            ps[:], xt[:, j, :, c0:c0 + 128], wt[:, j],
                            start=(pi == 0 and j == 0), stop=False,
                            perf_mode=DR,
                        )
                nc.tensor.matmul(
                    ps[:], sb["vones"][:], sb["bv8"][:],
                    start=False, stop=True, perf_mode=DR)
            def e():
                src = ps[:, 0:HPC * 64].rearrange("p (h m) -> p h m", m=64)
                if k < NSK8:
                    vf = W2.tile([128, HPC, 64], F16, tag="vf", name=f"vf{k}")
                    nc.vector.tensor_scalar_mul(vf[:], src, 0.0625)
                    d8 = v8[:, k, :, 0:64]
                    nc.vector.tensor_copy(d8, vf[:])
                    nc.vector.scalar_tensor_tensor(
                        dv8[:, k, :, 0:64], d8, -1.0, vf[:], MULT, ADD)
                else:
                    nc.vector.tensor_scalar_mul(
                        vp16[:, k - NSK8, :, 0:64], src, 0.0625)
            return m, e

        def proj_piece(sc, ms, half, ctx01, ctx2, ot):
            n0, nw = (0, 512) if half == 0 else (512, 256)
            ps = ACC.tile([128, nw], F32, tag="acc", name=f"ops{sc}_{ms}_{half}")
            def m():
                nc.tensor.matmul(ps[:], ctx01[:, ms * 128:(ms + 1) * 128],
                                 sb["wo01"][:, n0:n0 + nw],
                                 start=True, stop=False)
                nc.tensor.matmul(ps[:], ctx2[:, ms * 128:(ms + 1) * 128],
                                 sb["wo2"][:, n0:n0 + nw],
                                 start=False, stop=True)
            def e():
                nc.vector.tensor_copy(ot[:, n0:n0 + nw], ps[:])
                if half == 1:
                    r0 = (sc * 4 + ms) * 128
                    nc.sync.dma_start(out[r0:r0 + 128, :], ot[:])
            return m, e

        filler = deque()

        def extend_units(units):
            prev_e = None
            for m, e in units:
                filler.append(m)
                if prev_e is not None:
                    filler.append(prev_e)
                prev_e = e
            filler.append(prev_e)

        def pop(n=1):
            for _ in range(n):
                if filler:
                    filler.popleft()()

        def attention_block(sc, h, ctx01, ctx2, pops=None, lag=1):
            probs8 = probs16 = None
            if NSK8:
                probs8 = PR8.tile([128, NSK8, SQ], F8, tag="p8",
                                  name=f"p8_{sc}_{h}")
            if NSK8 < NSK:
                probs16 = PR16.tile([128, NSK - NSK8, SQ], F16, tag="p16",
                                    name=f"p16_{sc}_{h}")
            cps = CPS.tile([128, SQ], F32, tag="cps", name=f"cps{sc}_{h}")
            pv = [0]
            first = [True]

            def probsv_advance(bound):
                while pv[0] < NSK8 and pv[0] + 2 <= bound:
                    c = pv[0]
                    for t8 in (v8, dv8):
                        nc.tensor.matmul(
                            cps[:], t8[:, c:c + 2, h, :],
                            probs8[:, c:c + 2, :],
                            start=first[0],
                            stop=(c + 2 == NSK and t8 is dv8),
                            perf_mode=DR)
                        first[0] = False
                    pv[0] = c + 2
                while pv[0] >= NSK8 and pv[0] < bound:
                    c = pv[0]
                    nc.tensor.matmul(
                        cps[:], vp16[:, c - NSK8, h, :],
                        probs16[:, c - NSK8, :],
                        start=first[0], stop=(c + 1 == NSK))
                    first[0] = False
                    pv[0] = c + 1

            for gi, (g0, g1) in enumerate(GROUPS):
                pop(pops[gi] if pops else 1)
                n = g1 - g0
                pool = SPSB if n == 4 else SPSS
                sps = pool.tile([128, n * SQ], F32, tag=f"sps{n}",
                                name=f"sps{sc}_{h}_{gi}")
                for mk in range(g0, g1):
                    nc.tensor.matmul(
                        sps[:, (mk - g0) * SQ:(mk - g0 + 1) * SQ],
                        kT[h][:, mk * 128:(mk + 1) * 128],
                        qT[h][:, sc * SQ:(sc + 1) * SQ],
                        start=True, stop=True,
                    )
                dst = (probs8[:, g0:g1] if g1 <= NSK8
                       else probs16[:, g0 - NSK8:g1 - NSK8])
                nc.scalar.activation(dst, sps[:], EXP, scale=0.125,
                                     bias=ebias[:])
                if gi >= lag:
                    probsv_advance(GROUPS[gi - lag][1])
            pop(pops[len(GROUPS)] if pops else 2)
            probsv_advance(NSK)
            r = W2.tile([64, SQ], F32, tag="recip", name=f"r{sc}_{h}")
            nc.vector.reciprocal(r[:], cps[64:128, :])
            dst = ctx01[h * 64:(h + 1) * 64, :] if h < 2 else ctx2[:]
            nc.vector.tensor_tensor(dst, cps[0:64, :], r[:], MULT)

        warm = P1.tile([64, 512], F32R, tag="warm")
        nc.vector.memset(warm[:].bitcast(F32), 0.0)
        wps = ACC.tile([128, 512], F32, tag="acc", name="warmps")
        for _ in range(8):
            nc.tensor.matmul(wps[:], warm[:, 0:128], warm[:], start=True, stop=True)
        wact = P1.tile([64, 1], F16, tag="wact")
        nc.scalar.activation(wact[:], warm[:, 0:1].bitcast(F32), EXP,
                             scale=0.125, bias=ebias[0:64])

        qm, qe = qk_unit(0, 0)
        qm()
        qe()

        ctxs = {}
        for sc in range(NSQ):
            ctxs[sc] = (
                W2.tile([128, SQ], F32R, tag="ctx01", name=f"c01_{sc}"),
                W2.tile([64, SQ], F32R, tag="ctx2", name=f"c2_{sc}"),
            )
            for h in range(HPC):
                pops, lag = None, 1
                if sc == 0 and h == 0:
                    extend_units(
                        [qk_unit(0, 1), qk_unit(0, 2)]
                        + [v_unit(k) for k in range(0, 4)]
                        + [qk_unit(0, 3)]
                        + [v_unit(k) for k in range(4, 16)]
                        + [qk_unit(1, 0)]
                    )
                    pops, lag = [5, 8, 4, 6, 4, 8, 4, 1], 2
                elif sc == 0 and h == 1:
                    extend_units([qk_unit(1, 1), qk_unit(1, 2),
                                  qk_unit(1, 3), qk_unit(2, 0)])
                    pops = [2, 1, 2, 1, 1, 1, 0, 0]
                elif sc == 0 and h == 2:
                    extend_units([qk_unit(2, i) for i in range(1, NSQ)])
                    pops = [2, 1, 2, 1, 0, 0, 0, 0]
                attention_block(sc, h, *ctxs[sc], pops=pops, lag=lag)
            ot_tiles = [
                W2.tile([128, D], F32, tag="ot", name=f"ot{sc}_{ms}")
                for ms in range(4)
            ]
            extend_units([
                proj_piece(sc, ms, half, *ctxs[sc], ot_tiles[ms])
                for ms in range(4) for half in range(2)
            ])
        while filler:
            filler.popleft()()


_CACHE = {}


def _get_module():
    if "nc" not in _CACHE:
        _CACHE["nc"] = _build_module()
    return _CACHE["nc"]


def _split8(a):
    a8 = a.astype(E4M3)
    af = a8.astype(np.float32)
    da8 = ((a - af) * 16.0).astype(E4M3)
    a16 = (af / 16.0).astype(E4M3)
    return a8, da8, a16


def _pairchunk(a):
    n = a.shape[1]
    return np.ascontiguousarray(
        a.reshape(NOP, 2, 128, n).transpose(2, 0, 1, 3))


def make_in_maps(x, Wq, Wk, Wv, bq, bk, bv, Wo):
    f = np.float32
    in_maps = []
    vones = np.zeros((1, 2, 128), E4M3)
    vones[0, 0] = 1.0
    vones[0, 1] = 0.0625
    for c in range(NCORES):
        b = c // CORES_PER_BATCH
        hh = [HPC * (c % CORES_PER_BATCH) + i for i in range(HPC)]
        x8, dx8, x16 = (_pairchunk(t) for t in _split8(np.asarray(x[b]).T))
        wqk = np.stack(
            [np.concatenate([Wq[h], Wk[h]], axis=1) for h in hh]) * 16.0
        w8, dw8, w16 = (
            np.stack([_pairchunk(t[i]) for i in range(HPC)], axis=1)
            for t in _split8(wqk))
        wv_stack = np.concatenate(
            [Wv[h] for h in hh] + [np.zeros((D, 64), f)], axis=1) * 16.0
        v8, dv8, v16 = (_pairchunk(t) for t in _split8(wv_stack))
        bv_cat = np.concatenate([bv[h] for h in hh] + [np.zeros(64, f)]) * 16.0
        bv8 = np.zeros((1, 2, 256), E4M3)
        bv8[0, 0] = bv_cat.astype(E4M3)
        bv8[0, 1] = ((bv_cat - bv8[0, 0].astype(f)) * 16.0).astype(E4M3)
        in_maps.append({
            "x8": x8, "dx8": dx8, "x16": x16,
            "wqk8": w8, "wqk16": w16, "dwqk8": dw8,
            "wv8": v8, "wv16": v16, "dwv8": dv8,
            "bqk": np.ascontiguousarray(
                np.stack([np.concatenate([bq[h], bk[h]]) for h in hh], axis=1)
            ).astype(f, copy=False),
            "bv8": bv8, "vones": vones,
            "wo01": np.ascontiguousarray(
                Wo[hh[0] * DH:(hh[0] + 2) * DH, :]).astype(f, copy=False),
            "wo2": np.ascontiguousarray(
                Wo[hh[2] * DH:(hh[2] + 1) * DH, :]).astype(f, copy=False),
        })
    return in_maps


def gather(results, bo):
    out = np.empty((B, S, D), np.float32)
    for b in range(B):
        acc = results[b * CORES_PER_BATCH]["out"].astype(np.float32, copy=True)
        for c in range(b * CORES_PER_BATCH + 1, (b + 1) * CORES_PER_BATCH):
            acc += results[c]["out"]
        out[b] = acc + bo[None, :].astype(np.float32)
    return out


def kernel(x, Wq, Wk, Wv, bq, bk, bv, Wo, bo, c=0, **_unused):
    x, Wq, Wk, Wv, bq, bk, bv, Wo, bo = (
        np.asarray(a, np.float32) for a in (x, Wq, Wk, Wv, bq, bk, bv, Wo, bo)
    )
    nc = _get_module()
    in_maps = make_in_maps(x, Wq, Wk, Wv, bq, bk, bv, Wo)
    res = run_bass_kernel_spmd(nc, in_maps, list(range(NCORES)))
    return gather(res.results, bo)


# revision 18
# speedup vs baseline: 1.4202x; 1.2187x over previous
from collections import deque

import numpy as np
import ml_dtypes

import concourse.mybir as mybir
from concourse import bacc
from concourse.tile import TileContext
from concourse.bass_utils import run_bass_kernel_spmd

H, D, DH = 12, 768, 64
B, S = 2, 2048
NCORES = 8
CORES_PER_BATCH = 4
HPC = 3
SQ = 512
NSQ = S // SQ
NSK = S // 128
NOP = 3

GROUPS = tuple((2 * i, 2 * i + 2) for i in range(8))
NSK8 = 16
assert NSK8 % 2 == 0
EXPBIAS = -1.5

F32 = mybir.dt.float32
F32R = mybir.dt.float32r
F16 = mybir.dt.float16
F8 = mybir.dt.float8e4
DR = mybir.MatmulPerfMode.DoubleRow
ADD = mybir.AluOpType.add
MULT = mybir.AluOpType.mult
EXP = mybir.ActivationFunctionType.Exp
E4M3 = ml_dtypes.float8_e4m3fn


def _build_module():
    nc = bacc.Bacc("TRN2", target_bir_lowering=False, debug=False, num_devices=NCORES)
    dram = {}
    def P(name, shape, dt):
        dram[name] = nc.declare_dram_parameter(name, shape, dt, isOutput=False)
    P("x8", [128, NOP, 2, S], F8)
    P("dx8", [128, NOP, 2, S], F8)
    P("x16", [128, NOP, 2, S], F8)
    P("wqk8", [128, HPC, NOP, 2, 128], F8)
    P("wqk16", [128, HPC, NOP, 2, 128], F8)
    P("dwqk8", [128, HPC, NOP, 2, 128], F8)
    P("wv8", [128, NOP, 2, 256], F8)
    P("wv16", [128, NOP, 2, 256], F8)
    P("dwv8", [128, NOP, 2, 256], F8)
    P("bqk", [128, HPC], F32)
    P("bv8", [1, 2, 256], F8)
    P("vones", [1, 2, 128], F8)
    P("wo01", [128, D], F32R)
    P("wo2", [64, D], F32R)
    out = nc.declare_dram_parameter("out", [S, D], F32, isOutput=True)

    with TileContext(nc) as tc:
        _body(nc, tc, dram, out)
    nc.compile()
    return nc


def _body(nc, tc, dram, out):
    with (
        tc.tile_pool(name="persist", bufs=1) as P1,
        tc.tile_pool(name="work", bufs=4) as W2,
        tc.tile_pool(name="pr8", bufs=2) as PR8,
        tc.tile_pool(name="pr16", bufs=2) as PR16,
        tc.tile_pool(name="spsb", bufs=1, space="PSUM") as SPSB,
        tc.tile_pool(name="spss", bufs=1, space="PSUM") as SPSS,
        tc.tile_pool(name="cps", bufs=2, space="PSUM") as CPS,
        tc.tile_pool(name="acc", bufs=2, space="PSUM") as ACC,
    ):
        sb = {}
        for name in ("x8", "dx8", "x16", "wqk8", "wqk16", "dwqk8",
                     "wv8", "wv16", "dwv8", "bqk", "bv8", "vones",
                     "wo01", "wo2"):
            shape = list(dram[name].shape)
            dt = {"bqk": F32, "wo01": F32R, "wo2": F32R}.get(name, F8)
            sb[name] = P1.tile(shape, dt, tag=name, name=name)

        qT = [P1.tile([64, S], F32R, tag=f"qT{h}", name=f"qT{h}")
              for h in range(HPC)]
        kT = [P1.tile([64, S], F32R, tag=f"kT{h}", name=f"kT{h}")
              for h in range(HPC)]
        v8 = dv8 = vp16 = None
        if NSK8:
            v8 = P1.tile([128, NSK8, HPC, 128], F8, tag="v8", name="v8")
            dv8 = P1.tile([128, NSK8, HPC, 128], F8, tag="dv8", name="dv8")
        if NSK8 < NSK:
            vp16 = P1.tile([128, NSK - NSK8, HPC, 128], F16, tag="vp16",
                           name="vp16")

        dma = nc.sync.dma_start
        dma(sb["x8"][:, :, :, 0:SQ], dram["x8"][:, :, :, 0:SQ])
        dma(sb["wqk8"][:, 0], dram["wqk8"][:, 0])
        dma(sb["dx8"][:, :, :, 0:SQ], dram["dx8"][:, :, :, 0:SQ])
        dma(sb["wqk16"][:, 0], dram["wqk16"][:, 0])
        dma(sb["dwqk8"][:, 0], dram["dwqk8"][:, 0])
        dma(sb["x16"][:, :, :, 0:SQ], dram["x16"][:, :, :, 0:SQ])
        dma(sb["bqk"][:], dram["bqk"][:])

        def dma_x(sc):
            sl = np.s_[:, :, :, sc * SQ:(sc + 1) * SQ]
            dma(sb["x8"][sl], dram["x8"][sl])
            dma(sb["dx8"][sl], dram["dx8"][sl])
            dma(sb["x16"][sl], dram["x16"][sl])

        dma_x(1)
        for n in ("wv8", "wv16", "dwv8", "bv8", "vones"):
            dma(sb[n][:], dram[n][:])
        dma_x(2)
        dma(sb["wqk8"][:, 1:3], dram["wqk8"][:, 1:3])
        dma(sb["wqk16"][:, 1:3], dram["wqk16"][:, 1:3])
        dma(sb["dwqk8"][:, 1:3], dram["dwqk8"][:, 1:3])
        dma_x(3)
        dma(sb["wo01"][:], dram["wo01"][:])
        dma(sb["wo2"][:], dram["wo2"][:])
        ebias = P1.tile([128, 1], F32, tag="ebias")
        nc.vector.memset(ebias[:], EXPBIAS)
        if v8 is not None:
            nc.gpsimd.memset(v8[:, :, :, 64:128], 1.0)
            nc.gpsimd.memset(dv8[:, :, :, 64:128], 0.0)
        if vp16 is not None:
            nc.gpsimd.memset(vp16[:, :, :, 64:128], 1.0)


        def qk_unit(h, sc):
            ps = ACC.tile([128, SQ], F32, tag="acc", name=f"qkps{h}_{sc}")
            s0 = sc * SQ
            passes = (
                (sb["wqk8"], sb["x8"]),
                (sb["wqk16"], sb["dx8"]),
                (sb["dwqk8"], sb["x16"]),
            )
            def m():
                for pi, (wt, xt) in enumerate(passes):
                    for j in range(NOP):
                        nc.tensor.matmul(
                            ps[:], wt[:, h, j], xt[:, j, :, s0:s0 + SQ],
                            start=(pi == 0 and j == 0),
                            stop=(pi == 2 and j == NOP - 1),
                            perf_mode=DR,
                        )
            def e():
                nc.vector.scalar_tensor_tensor(
                    qT[h][:, s0:s0 + SQ], ps[0:64, :], 0.0625,
                    sb["bqk"][0:64, h:h + 1].to_broadcast([64, SQ]),
                    MULT, ADD)
                nc.vector.scalar_tensor_tensor(
                    kT[h][:, s0:s0 + SQ], ps[64:128, :], 0.0625,
                    sb["bqk"][64:128, h:h + 1].to_broadcast([64, SQ]),
                    MULT, ADD)
            return m, e

        def v_unit(k):
            ps = ACC.tile([128, 256], F32, tag="acc", name=f"vps{k}")
            c0 = k * 128
            passes = (
                (sb["x8"], sb["wv8"]),
                (sb["dx8"], sb["wv16"]),
                (sb["x16"], sb["dwv8"]),
            )
            def m():
                for pi, (xt, wt) in enumerate(passes):
                    for j in range(NOP):
                        nc.tensor.matmul(
                            ps[:], xt[:, j, :, c0:c0 + 128], wt[:, j],
                            start=(pi == 0 and j == 0), stop=False,
                            perf_mode=DR,
                        )
                nc.tensor.matmul(
                    ps[:], sb["vones"][:], sb["bv8"][:],
                    start=False, stop=True, perf_mode=DR)
            def e():
                src = ps[:, 0:HPC * 64].rearrange("p (h m) -> p h m", m=64)
                if k < NSK8:
                    vf = W2.tile([128, HPC, 64], F16, tag="vf", name=f"vf{k}")
                    nc.vector.tensor_scalar_mul(vf[:], src, 0.0625)
                    d8 = v8[:, k, :, 0:64]
                    nc.vector.tensor_copy(d8, vf[:])
                    nc.vector.scalar_tensor_tensor(
                        dv8[:, k, :, 0:64], d8, -1.0, vf[:], MULT, ADD)
                else:
                    nc.vector.tensor_scalar_mul(
                        vp16[:, k - NSK8, :, 0:64], src, 0.0625)
            return m, e

        def proj_piece(sc, ms, half, ctx01, ctx2, ot):
            n0, nw = (0, 512) if half == 0 else (512, 256)
            ps = ACC.tile([128, nw], F32, tag="acc", name=f"ops{sc}_{ms}_{half}")
            def m():
                nc.tensor.matmul(ps[:], ctx01[:, ms * 128:(ms + 1) * 128],
                                 sb["wo01"][:, n0:n0 + nw],
                                 start=True, stop=False)
                nc.tensor.matmul(ps[:], ctx2[:, ms * 128:(ms + 1) * 128],
                                 sb["wo2"][:, n0:n0 + nw],
                                 start=False, stop=True)
            def e():
                if sc == NSQ - 1 and half == 0:
                    nc.scalar.activation(ot[:, n0:n0 + nw], ps[:],
                                         mybir.ActivationFunctionType.Copy)
                else:
                    nc.vector.tensor_copy(ot[:, n0:n0 + nw], ps[:])
                r0 = (sc * 4 + ms) * 128
                nc.sync.dma_start(out[r0:r0 + 128, n0:n0 + nw],
                                  ot[:, n0:n0 + nw])
            return m, e

        filler = deque()

        def extend_units(units):
            prev_e = None
            for m, e in units:
                filler.append(m)
                if prev_e is not None:
                    filler.append(prev_e)
                prev_e = e
            filler.append(prev_e)

        def pop(n=1):
            for _ in range(n):
                if filler:
                    filler.popleft()()


        class BlockState:

            def __init__(self, sc, h, ctx01, ctx2):
                self.sc, self.h = sc, h
                self.ctx01, self.ctx2 = ctx01, ctx2
                self.probs8 = self.probs16 = None
                if NSK8:
                    self.probs8 = PR8.tile([128, NSK8, SQ], F8, tag="p8",
                                           name=f"p8_{sc}_{h}")
                if NSK8 < NSK:
                    self.probs16 = PR16.tile([128, NSK - NSK8, SQ], F16,
                                             tag="p16", name=f"p16_{sc}_{h}")
                self.cps = CPS.tile([128, SQ], F32, tag="cps",
                                    name=f"cps{sc}_{h}")
                self.pv = 0
                self.first = True

            def advance(self, bound):
                h = self.h
                while self.pv < NSK8 and self.pv + 2 <= bound:
                    c = self.pv
                    for t8 in (v8, dv8):
                        nc.tensor.matmul(
                            self.cps[:], t8[:, c:c + 2, h, :],
                            self.probs8[:, c:c + 2, :],
                            start=self.first,
                            stop=(c + 2 == NSK and t8 is dv8),
                            perf_mode=DR)
                        self.first = False
                    self.pv = c + 2
                while self.pv >= NSK8 and self.pv < bound:
                    c = self.pv
                    nc.tensor.matmul(
                        self.cps[:], vp16[:, c - NSK8, h, :],
                        self.probs16[:, c - NSK8, :],
                        start=self.first, stop=(c + 1 == NSK))
                    self.first = False
                    self.pv = c + 1

            def finish(self):
                self.advance(NSK)
                r = W2.tile([64, SQ], F32, tag="recip",
                            name=f"r{self.sc}_{self.h}")
                nc.vector.reciprocal(r[:], self.cps[64:128, :])
                dst = (self.ctx01[self.h * 64:(self.h + 1) * 64, :]
                       if self.h < 2 else self.ctx2[:])
                nc.vector.tensor_tensor(dst, self.cps[0:64, :], r[:], MULT)

        def attention_block(sc, h, ctx01, ctx2, prev, pops=None,
                            prev_bounds=None, self_pv=False):
            st = BlockState(sc, h, ctx01, ctx2)
            if prev_bounds is None:
                prev_bounds = tuple(2 * i for i in range(8))
            for gi, (g0, g1) in enumerate(GROUPS):
                pop(pops[gi] if pops else 1)
                pool = SPSB if gi % 2 == 0 else SPSS
                sps = pool.tile([128, 2 * SQ], F32, tag="sps",
                                name=f"sps{sc}_{h}_{gi}")
                for mk in range(g0, g1):
                    nc.tensor.matmul(
                        sps[:, (mk - g0) * SQ:(mk - g0 + 1) * SQ],
                        kT[h][:, mk * 128:(mk + 1) * 128],
                        qT[h][:, sc * SQ:(sc + 1) * SQ],
                        start=True, stop=True,
                    )
                dst = (st.probs8[:, g0:g1] if g1 <= NSK8
                       else st.probs16[:, g0 - NSK8:g1 - NSK8])
                nc.scalar.activation(dst, sps[:], EXP, scale=0.125,
                                     bias=ebias[:])
                if prev is not None:
                    prev.advance(prev_bounds[gi])
                    if prev_bounds[gi] >= NSK and prev.pv >= NSK:
                        prev.finish()
                        prev = None
                if self_pv and gi >= 1:
                    st.advance(g0)
            pop(pops[len(GROUPS)] if pops else 0)
            if prev is not None:
                prev.finish()
            return st

        warm = P1.tile([64, 512], F32R, tag="warm")
        nc.vector.memset(warm[:].bitcast(F32), 0.0)
        wps = ACC.tile([128, 512], F32, tag="acc", name="warmps")
        for _ in range(8):
            nc.tensor.matmul(wps[:], warm[:, 0:128], warm[:], start=True, stop=True)
        wact = P1.tile([64, 1], F16, tag="wact")
        nc.scalar.activation(wact[:], warm[:, 0:1].bitcast(F32), EXP,
                             scale=0.125, bias=ebias[0:64])

        qm, qe = qk_unit(0, 0)
        qm()
        qe()

        ctxs = {
            sc: (
                W2.tile([128, SQ], F32R, tag="ctx01", name=f"c01_{sc}"),
                W2.tile([64, SQ], F32R, tag="ctx2", name=f"c2_{sc}"),
            )
            for sc in range(NSQ)
        }
        ot_tiles = {
            (sc, ms): W2.tile([128, D], F32, tag="ot", name=f"ot{sc}_{ms}")
            for sc in range(NSQ) for ms in range(4)
        }
        POPS = {
            (0, 0): [3, 3, 3, 3, 3, 3, 3, 3, 0],
            (0, 1): [3, 3, 3, 3, 3, 3, 3, 3, 0],
            (0, 2): [1, 1, 1, 1, 1, 1, 0, 0, 0],
        }
        prev = None
        for sc in range(NSQ):
            for h in range(HPC):
                if sc == 0 and h == 0:
                    extend_units(
                        [qk_unit(0, 1), v_unit(0), v_unit(1), qk_unit(0, 2),
                         v_unit(2), v_unit(3), v_unit(4), v_unit(5),
                         qk_unit(0, 3), v_unit(6), v_unit(7), qk_unit(1, 0)]
                    )
                elif sc == 0 and h == 1:
                    extend_units(
                        [qk_unit(1, 1)] + [v_unit(8), v_unit(9)]
                        + [qk_unit(1, 2)] + [v_unit(10), v_unit(11)]
                        + [qk_unit(1, 3)]
                        + [v_unit(k) for k in range(12, 16)]
                        + [qk_unit(2, 0)]
                    )
                elif sc == 0 and h == 2:
                    extend_units([qk_unit(2, i) for i in range(1, NSQ)])
                last = sc == NSQ - 1 and h == HPC - 1
                prev = attention_block(
                    sc, h, *ctxs[sc], prev, pops=POPS.get((sc, h)),
                    prev_bounds=(4, 6, 8, 10, 12, 14, 16, 16) if last
                    else None,
                    self_pv=last)
            if sc >= 1:
                extend_units([
                    proj_piece(sc - 1, ms, half, *ctxs[sc - 1],
                               ot_tiles[(sc - 1, ms)])
                    for ms in range(4) for half in range(2)
                ])
        prev.finish()
        extend_units([
            proj_piece(NSQ - 1, ms, half, *ctxs[NSQ - 1],
                       ot_tiles[(NSQ - 1, ms)])
            for ms in range(4) for half in range(2)
        ])
        while filler:
            filler.popleft()()


_CACHE = {}


def _get_module():
    if "nc" not in _CACHE:
        _CACHE["nc"] = _build_module()
    return _CACHE["nc"]


def _split8(a):
    a8 = a.astype(E4M3)
    af = a8.astype(np.float32)
    da8 = ((a - af) * 16.0).astype(E4M3)
    a16 = (af / 16.0).astype(E4M3)
    return a8, da8, a16


def _pairchunk(a):
    n = a.shape[1]
    return np.ascontiguousarray(
        a.reshape(NOP, 2, 128, n).transpose(2, 0, 1, 3))


def make_in_maps(x, Wq, Wk, Wv, bq, bk, bv, Wo):
    f = np.float32
    in_maps = []
    vones = np.zeros((1, 2, 128), E4M3)
    vones[0, 0] = 1.0
    vones[0, 1] = 0.0625
    for c in range(NCORES):
        b = c // CORES_PER_BATCH
        hh = [HPC * (c % CORES_PER_BATCH) + i for i in range(HPC)]
        x8, dx8, x16 = (_pairchunk(t) for t in _split8(np.asarray(x[b]).T))
        wqk = np.stack(
            [np.concatenate([Wq[h], Wk[h]], axis=1) for h in hh]) * 16.0
        w8, dw8, w16 = (
            np.stack([_pairchunk(t[i]) for i in range(HPC)], axis=1)
            for t in _split8(wqk))
        wv_stack = np.concatenate(
            [Wv[h] for h in hh] + [np.zeros((D, 64), f)], axis=1) * 16.0
        v8, dv8, v16 = (_pairchunk(t) for t in _split8(wv_stack))
        bv_cat = np.concatenate([bv[h] for h in hh] + [np.zeros(64, f)]) * 16.0
        bv8 = np.zeros((1, 2, 256), E4M3)
        bv8[0, 0] = bv_cat.astype(E4M3)
        bv8[0, 1] = ((bv_cat - bv8[0, 0].astype(f)) * 16.0).astype(E4M3)
        in_maps.append({
            "x8": x8, "dx8": dx8, "x16": x16,
            "wqk8": w8, "wqk16": w16, "dwqk8": dw8,
            "wv8": v8, "wv16": v16, "dwv8": dv8,
            "bqk": np.ascontiguousarray(
                np.stack([np.concatenate([bq[h], bk[h]]) for h in hh], axis=1)
            ).astype(f, copy=False),
            "bv8": bv8, "vones": vones,
            "wo01": np.ascontiguousarray(
                Wo[hh[0] * DH:(hh[0] + 2) * DH, :]).astype(f, copy=False),
            "wo2": np.ascontiguousarray(
                Wo[hh[2] * DH:(hh[2] + 1) * DH, :]).astype(f, copy=False),
        })
    return in_maps


def gather(results, bo):
    out = np.empty((B, S, D), np.float32)
    for b in range(B):
        acc = results[b * CORES_PER_BATCH]["out"].astype(np.float32, copy=True)
        for c in range(b * CORES_PER_BATCH + 1, (b + 1) * CORES_PER_BATCH):
            acc += results[c]["out"]
        out[b] = acc + bo[None, :].astype(np.float32)
    return out


def kernel(x, Wq, Wk, Wv, bq, bk, bv, Wo, bo, c=0, **_unused):
    x, Wq, Wk, Wv, bq, bk, bv, Wo, bo = (
        np.asarray(a, np.float32) for a in (x, Wq, Wk, Wv, bq, bk, bv, Wo, bo)
    )
    nc = _get_module()
    in_maps = make_in_maps(x, Wq, Wk, Wv, bq, bk, bv, Wo)
    res = run_bass_kernel_spmd(nc, in_maps, list(range(NCORES)))
    return gather(res.results, bo)
